# revision 2
# baseline (speedup 1.0000x reference)
"""Bass/Trainium2 kernel for nn_BlockForNormalWindow (windowed-attention
transformer block), data-parallel over batch across 8 NeuronCores.

v2: bf16 compute, SBUF-resident composite qTa/kTa score operands
(rel-pos rows + q/k rows stacked on the contract dim), per-window
score/AV matmuls, software-pipelined phase interleaving (qkv+rel per
head; MLP groups interleaved into attention groups).
"""
import sys
sys.path.insert(0, '/opt/trn_rl_repo')

import numpy as np
import ml_dtypes
import concourse.bass as bass
import concourse.mybir as mybir
import concourse.tile as tile
from concourse import bacc
from concourse.bass_utils import run_bass_kernel_spmd
from concourse.masks import make_identity

F32 = mybir.dt.float32
BF16 = mybir.dt.bfloat16
FP8 = mybir.dt.float8e4
AF = mybir.ActivationFunctionType
ALU = mybir.AluOpType
BF = ml_dtypes.bfloat16
F8 = ml_dtypes.float8_e4m3

B, H, W = 8, 64, 64
DIM, NH, WS = 384, 6, 14
HD = DIM // NH               # 64
MLP = 4 * DIM                # 1536
EPS = 1e-5
SCALE = HD ** -0.5
GRID = 70                    # padded grid side
NW = 25                      # windows
N = WS * WS                  # 196 tokens per window
NTOK = NW * N                # 4900
NVAL = H * W                 # 4096

# window groups for qkv/attention: 12 groups of 2 windows + 1 single
GROUPS = [(2 * g, 2) for g in range(12)] + [(24, 1)]
# E-group eg emitted after D-group g
E_AFTER = {2: [0], 5: [1, 2], 7: [3, 4], 9: [5, 6], 12: [7]}
# v-chunk ranges computed per BC head iteration
VC_PER_H = [9, 9, 8, 8, 8, 8]


def _win_origin(w):
    return (w // 5) * 14 * GRID + (w % 5) * 14


def _ap(t, offset_elems, dims):
    """AP at partition range of t (sliced), offset in elements, free dims."""
    a = t[:, 0:1]
    return bass.AP(tensor=a.tensor, offset=a.offset + offset_elems,
                   ap=[a.ap[0]] + dims)


def _app(t, p0, p1, offset_elems, dims):
    """Like _ap but with partition slice [p0:p1]."""
    a = t[p0:p1, 0:1]
    return bass.AP(tensor=a.tensor, offset=a.offset + offset_elems,
                   ap=[a.ap[0]] + dims)


def build_bass():
    nc = bacc.Bacc("TRN2", target_bir_lowering=False, debug=False)

    x_in = nc.dram_tensor("x", [NVAL, DIM], F32, kind="ExternalInput")
    wqk_in = nc.dram_tensor("wqk", [DIM, 2 * DIM], BF16, kind="ExternalInput")
    bq_in = nc.dram_tensor("bq", [NH * 128], F32, kind="ExternalInput")
    wv_in = nc.dram_tensor("wv", [DIM, DIM], BF16, kind="ExternalInput")
    relm_in = nc.dram_tensor("relm", [128, 2 * N], BF16, kind="ExternalInput")
    kinit_in = nc.dram_tensor("kinit", [64, NTOK], BF16, kind="ExternalInput")
    qinit_in = nc.dram_tensor("qinit", [64, NTOK], BF16, kind="ExternalInput")
    wp_in = nc.dram_tensor("wp", [DIM, DIM], FP8, kind="ExternalInput")
    bp_in = nc.dram_tensor("bp", [DIM], F32, kind="ExternalInput")
    w1_in = nc.dram_tensor("w1", [DIM, MLP], FP8, kind="ExternalInput")
    b1_in = nc.dram_tensor("b1", [MLP], F32, kind="ExternalInput")
    w2_in = nc.dram_tensor("w2", [MLP, DIM], FP8, kind="ExternalInput")
    b2_in = nc.dram_tensor("b2", [DIM], F32, kind="ExternalInput")
    out_d = nc.dram_tensor("out", [NVAL, DIM], F32, kind="ExternalOutput")

    v_d = nc.dram_tensor("v_d", [NTOK, NH * 66], BF16)
    y_d = nc.dram_tensor("y_d", [64 * GRID, DIM], FP8)

    with tile.TileContext(nc) as tc:
      with tc.tile_pool(name="singles", bufs=1) as singles:
        ident_f = singles.tile([128, 128], F32)
        make_identity(nc, ident_f[:])
        ident = singles.tile([128, 128], BF16)
        nc.vector.tensor_copy(out=ident[:], in_=ident_f[:])

        eps_t = singles.tile([128, 1], F32)
        nc.vector.memset(eps_t[:], EPS)
        ones64 = singles.tile([1, 64], BF16)
        nc.vector.memset(ones64[:], 1.0)
        ones98 = singles.tile([98, 1], BF16)
        nc.vector.memset(ones98[:], 1.0)

        bq_t = singles.tile([128, NH], F32)
        bp_t = singles.tile([128, 3], F32)
        b1_t = singles.tile([128, 12], F32)
        b2_t = singles.tile([128, 3], F32)
        wqk_t = singles.tile([128, 3, 2 * DIM], BF16)
        wv_t = singles.tile([128, 3, DIM], BF16)
        wp_t = singles.tile([128, 3, DIM], FP8)
        relm_t = singles.tile([128, 2 * N], BF16)

        # resident composite score operands per head:
        #   rows 0-13 relh/kpat-h, 14-31 zero, 32-45 relw/kpat-w, 46-63 zero,
        #   rows 64-127 q/k
        with tc.tile_pool(name="pQK", bufs=1) as pQK:
            qTa = [pQK.tile([128, NTOK], BF16, tag=f"qTa{h}", name=f"qTa{h}")
                   for h in range(NH)]
            kTa = [pQK.tile([128, NTOK], BF16, tag=f"kTa{h}", name=f"kTa{h}")
                   for h in range(NH)]
            def load_singles():
                nc.sync.dma_start(out=bq_t[:], in_=bq_in.ap().rearrange("(m p) -> p m", p=128))
                nc.sync.dma_start(out=bp_t[:], in_=bp_in.ap().rearrange("(m p) -> p m", p=128))
                nc.sync.dma_start(out=b1_t[:], in_=b1_in.ap().rearrange("(m p) -> p m", p=128))
                nc.sync.dma_start(out=b2_t[:], in_=b2_in.ap().rearrange("(m p) -> p m", p=128))
                nc.sync.dma_start(out=wqk_t[:], in_=wqk_in.ap().rearrange("(kc p) n -> p kc n", p=128))
                nc.sync.dma_start(out=wv_t[:], in_=wv_in.ap().rearrange("(kc p) n -> p kc n", p=128))
                nc.sync.dma_start(out=wp_t[:], in_=wp_in.ap().rearrange("(kc p) n -> p kc n", p=128))
                nc.sync.dma_start(out=relm_t[:], in_=relm_in.ap())
                for h in range(NH):
                    nc.sync.dma_start(out=qTa[h][0:64, :], in_=qinit_in.ap())
                    nc.sync.dma_start(out=kTa[h][0:64, :], in_=kinit_in.ap())

            # ---- Phase A: LN1 + transpose into hT, then B/C per head ----
            with tc.tile_pool(name="pHT", bufs=1) as pHT:
                hT = pHT.tile([128, 3 * NTOK], BF16, tag="hT", name="hT")
                # zero the pad region of the 70x70 grid
                for kc in range(3):
                    nc.gpsimd.memset(_ap(hT, kc * NTOK + 64 * GRID,
                                         [[1, NTOK - 64 * GRID]]), 0.0)
                    nc.gpsimd.memset(_ap(hT, kc * NTOK + 64,
                                         [[GRID, 64], [1, 6]]), 0.0)

                with tc.tile_pool(name="pA", bufs=4) as pA, \
                     tc.tile_pool(name="pA_ps", bufs=2, space="PSUM") as pA_ps:
                    for t in range(32):
                        xt = pA.tile([128, DIM], F32, tag="xt")
                        nc.sync.dma_start(out=xt[:], in_=x_in[t * 128:(t + 1) * 128, :])
                        stats = pA.tile([128, 6], F32, tag="st")
                        nc.vector.bn_stats(out=stats[:], in_=xt[:])
                        mv = pA.tile([128, 2], F32, tag="mv")
                        nc.vector.bn_aggr(out=mv[:], in_=stats[:])
                        rstd = pA.tile([128, 1], F32, tag="rstd")
                        nc.scalar.activation(out=rstd[:], in_=mv[:, 1:2], func=AF.Sqrt,
                                             bias=eps_t[:], scale=1.0)
                        nc.vector.reciprocal(out=rstd[:], in_=rstd[:])
                        nmr = pA.tile([128, 1], F32, tag="nmr")
                        nc.vector.scalar_tensor_tensor(out=nmr[:], in0=mv[:, 0:1],
                                                       scalar=-1.0, in1=rstd[:],
                                                       op0=ALU.mult, op1=ALU.mult)
                        hn = pA.tile([128, DIM], BF16, tag="hn")
                        nc.scalar.activation(out=hn[:], in_=xt[:], func=AF.Identity,
                                             bias=nmr[:], scale=rstd[:])
                        pt = pA_ps.tile([128, DIM], BF16, tag="tra")
                        for c in range(3):
                            nc.tensor.transpose(pt[:, c * 128:(c + 1) * 128],
                                                hn[:, c * 128:(c + 1) * 128], ident[:])
                        dst = _ap(hT, 2 * t * GRID, [[NTOK, 3], [GRID, 2], [1, 64]])
                        if t % 2 == 0:
                            nc.vector.tensor_copy(out=dst, in_=pt[:])
                        else:
                            nc.scalar.copy(out=dst, in_=pt[:])

                load_singles()

                # ---- Phase B+C: qkv + rel, per head, v interleaved ----
                vchunk = 0
                with tc.tile_pool(name="pB", bufs=3) as pB, \
                     tc.tile_pool(name="pB_ps", bufs=2, space="PSUM") as pB_ps, \
                     tc.tile_pool(name="pC_ps", bufs=3, space="PSUM") as pC_ps, \
                     tc.tile_pool(name="pV_ps", bufs=2, space="PSUM") as pV_ps:
                    for h in range(NH):
                        # qk for all groups
                        for (w0, nwin) in GROUPS:
                            plen = nwin * N
                            g0 = w0 * N
                            o0 = _win_origin(w0)
                            if nwin == 2:
                                dims = [[_win_origin(w0 + 1) - o0, 2], [GRID, 14], [1, 14]]
                            else:
                                dims = [[GRID, 14], [1, 14]]
                            ps = pB_ps.tile([128, 392], F32, tag="qk")
                            for kc in range(3):
                                nc.tensor.matmul(ps[:, 0:plen],
                                                 wqk_t[:, kc, h * 128:(h + 1) * 128],
                                                 _ap(hT, kc * NTOK + o0, dims),
                                                 start=(kc == 0), stop=(kc == 2))
                            # k rows 64-127 -> kTa (same partitions, Act)
                            nc.scalar.copy(out=kTa[h][64:128, g0:g0 + plen],
                                           in_=ps[64:128, 0:plen])
                            # q rows 0-63 + bias -> qTa rows 64-127 (DVE shift)
                            nc.vector.tensor_scalar(out=qTa[h][64:128, g0:g0 + plen],
                                                    in0=ps[0:64, 0:plen],
                                                    scalar1=bq_t[0:64, h:h + 1],
                                                    scalar2=None, op0=ALU.add)
                        # rel rows for this head
                        for rc in range(14):
                            cps = pC_ps.tile([46, 350], F32, tag="c")
                            nc.tensor.matmul(cps[0:14, :],
                                             relm_t[64:128, rc * 14:(rc + 1) * 14],
                                             _app(qTa[h], 64, 128, rc * 14,
                                                  [[196, 25], [1, 14]]),
                                             start=True, stop=True)
                            nc.tensor.matmul(cps[32:46, :],
                                             relm_t[64:128, 196 + rc * 14:196 + (rc + 1) * 14],
                                             _app(qTa[h], 64, 128, rc,
                                                  [[196, 25], [14, 14]]),
                                             start=True, stop=True)
                            dsth = _app(qTa[h], 0, 14, rc * 14, [[196, 25], [1, 14]])
                            dstw = _app(qTa[h], 32, 46, rc, [[196, 25], [14, 14]])
                            if rc % 2 == 0:
                                nc.scalar.copy(out=dsth, in_=cps[0:14, :])
                                nc.vector.tensor_copy(out=dstw, in_=cps[32:46, :])
                            else:
                                nc.vector.tensor_copy(out=dsth, in_=cps[0:14, :])
                                nc.scalar.copy(out=dstw, in_=cps[32:46, :])
                        # v chunks interleaved
                        for _ in range(VC_PER_H[h]):
                            c = vchunk
                            vchunk += 1
                            w = c // 2
                            ov = _win_origin(w) + (c % 2) * 7 * GRID
                            vps = pV_ps.tile([98, DIM], F32, tag="v")
                            hstage = pB.tile([128, 3, 98], BF16, tag="hstage")
                            for kc in range(3):
                                nc.gpsimd.tensor_copy(
                                    out=hstage[:, kc, :],
                                    in_=_ap(hT, kc * NTOK + ov, [[GRID, 7], [1, 14]]))
                            for kc in range(3):
                                nc.tensor.matmul(vps[:], hstage[:, kc, :],
                                                 wv_t[:, kc, :],
                                                 start=(kc == 0), stop=(kc == 2))
                            vsb = pB.tile([98, NH * 66], BF16, tag="vsb")
                            if c < 3:
                                nc.vector.memset(_ap(vsb, 64, [[66, 6], [1, 1]]), 1.0)
                                nc.vector.memset(_ap(vsb, 65, [[66, 6], [1, 1]]), 0.0)
                            dstv = _ap(vsb, 0, [[66, 6], [1, 64]])
                            if c % 2 == 0:
                                nc.scalar.copy(out=dstv, in_=vps[:])
                            else:
                                nc.vector.tensor_copy(out=dstv, in_=vps[:])
                            nc.sync.dma_start(out=v_d[c * 98:(c + 1) * 98, :], in_=vsb[:])

            # ---- Phase D+E interleaved ----
            with tc.tile_pool(name="pW2", bufs=1) as pW2:
                w1_t = pW2.tile([128, 3, MLP], FP8)
                nc.sync.dma_start(out=w1_t[:], in_=w1_in.ap().rearrange("(kc p) n -> p kc n", p=128))
                w2_t = pW2.tile([128, 12, DIM], FP8)
                nc.sync.dma_start(out=w2_t[:], in_=w2_in.ap().rearrange("(kc p) n -> p kc n", p=128))

                with tc.tile_pool(name="pD", bufs=2) as pD, \
                     tc.tile_pool(name="pDv", bufs=2) as pDv, \
                     tc.tile_pool(name="pDa", bufs=2) as pDa, \
                     tc.tile_pool(name="pDe", bufs=4) as pDe, \
                     tc.tile_pool(name="pDy", bufs=1) as pDy, \
                     tc.tile_pool(name="pE", bufs=3) as pE, \
                     tc.tile_pool(name="pEz", bufs=1) as pEz, \
                     tc.tile_pool(name="pEh", bufs=1) as pEh, \
                     tc.tile_pool(name="pEg", bufs=1) as pEg, \
                     tc.tile_pool(name="psBig", bufs=3, space="PSUM") as psBig, \
                     tc.tile_pool(name="psO", bufs=1, space="PSUM") as psO, \
                     tc.tile_pool(name="psT", bufs=1, space="PSUM") as psT, \
                     tc.tile_pool(name="psF", bufs=3, space="PSUM") as psF:

                    def emit_E(eg):
                        zts = []
                        h2T = pEh.tile([128, 3 * 512], FP8, tag="h2T")
                        for tt in range(4):
                            t = eg * 4 + tt
                            xe = pE.tile([128, DIM], F32, tag="xe")
                            nc.sync.dma_start(out=xe[:], in_=x_in[t * 128:(t + 1) * 128, :])
                            ye = pE.tile([128, DIM], FP8, tag="ye")
                            ya2 = y_d.ap()
                            ysrc2 = bass.AP(tensor=ya2.tensor,
                                            offset=2 * t * GRID * DIM,
                                            ap=[[GRID * DIM, 2], [DIM, 64], [1, DIM]])
                            nc.sync.dma_start(out=ye[:], in_=ysrc2)
                            ze = pEz.tile([128, DIM], F32, tag=f"z{tt}")
                            nc.vector.tensor_tensor(out=ze[:], in0=xe[:], in1=ye[:],
                                                    op=ALU.add)
                            zts.append(ze)
                            stats = pE.tile([128, 6], F32, tag="ste")
                            nc.vector.bn_stats(out=stats[:], in_=ze[:])
                            mv = pE.tile([128, 2], F32, tag="mve")
                            nc.vector.bn_aggr(out=mv[:], in_=stats[:])
                            rstd = pE.tile([128, 1], F32, tag="rstde")
                            nc.scalar.activation(out=rstd[:], in_=mv[:, 1:2], func=AF.Sqrt,
                                                 bias=eps_t[:], scale=1.0)
                            nc.vector.reciprocal(out=rstd[:], in_=rstd[:])
                            nmr = pE.tile([128, 1], F32, tag="nmre")
                            nc.vector.scalar_tensor_tensor(out=nmr[:], in0=mv[:, 0:1],
                                                           scalar=-1.0, in1=rstd[:],
                                                           op0=ALU.mult, op1=ALU.mult)
                            hn = pE.tile([128, DIM], BF16, tag="hne")
                            nc.scalar.activation(out=hn[:], in_=ze[:], func=AF.Identity,
                                                 bias=nmr[:], scale=rstd[:])
                            pt = psT.tile([128, DIM], BF16, tag="tr")
                            for c in range(3):
                                nc.tensor.transpose(pt[:, c * 128:(c + 1) * 128],
                                                    hn[:, c * 128:(c + 1) * 128], ident[:])
                            dst = _ap(h2T, tt * 128, [[512, 3], [1, 128]])
                            nc.vector.tensor_copy(out=dst, in_=pt[:])
                        gt = pEg.tile([128, 12, 512], FP8, tag="gt")
                        for m in range(12):
                            f1 = psF.tile([128, 512], F32, tag="fc")
                            nc.tensor.matmul(f1[:], w1_t[:, 0:2, m * 128:(m + 1) * 128],
                                             _ap(h2T, 0, [[512, 2], [1, 512]]),
                                             start=True, stop=False,
                                             perf_mode=mybir.MatmulPerfMode.DoubleRow)
                            nc.tensor.matmul(f1[:], w1_t[:, 2, m * 128:(m + 1) * 128],
                                             _ap(h2T, 2 * 512, [[1, 512]]),
                                             start=False, stop=True)
                            nc.scalar.activation(out=gt[:, m, :], in_=f1[:], func=AF.Gelu,
                                                 bias=b1_t[:, m:m + 1], scale=1.0)
                        o2 = []
                        for m in range(3):
                            f2 = psF.tile([128, 512], F32, tag="fc")
                            for kc in range(6):
                                nc.tensor.matmul(f2[:], w2_t[:, 2 * kc:2 * kc + 2, m * 128:(m + 1) * 128],
                                                 gt[:, 2 * kc:2 * kc + 2, :],
                                                 start=(kc == 0), stop=(kc == 5),
                                                 perf_mode=mybir.MatmulPerfMode.DoubleRow)
                            om = pEh.tile([128, 512], BF16, tag=f"o2T{m}")
                            nc.vector.tensor_scalar(out=om[:], in0=f2[:],
                                                    scalar1=b2_t[:, m:m + 1],
                                                    scalar2=None, op0=ALU.add)
                            o2.append(om)
                        for tt in range(4):
                            t = eg * 4 + tt
                            pt = psT.tile([128, DIM], BF16, tag="tr")
                            for c in range(3):
                                nc.tensor.transpose(pt[:, c * 128:(c + 1) * 128],
                                                    o2[c][:, tt * 128:(tt + 1) * 128],
                                                    ident[:])
                            oe = pE.tile([128, DIM], F32, tag="oe")
                            nc.vector.tensor_tensor(out=oe[:], in0=zts[tt][:],
                                                    in1=pt[:], op=ALU.add)
                            nc.sync.dma_start(out=out_d[t * 128:(t + 1) * 128, :], in_=oe[:])

                    # software-pipelined D: stage A(g,h) = st+exp; stage B(g,h)
                    # = Z/recip/AV/bcast/mult one head behind; proj one group
                    # behind, so the in-order PE never waits on Act/DVE.
                    NG = len(GROUPS)
                    pend = []       # (gi, h, ets, vt, attnT) awaiting stage B
                    gstate = {}     # gi -> (plen, vt, attnT)

                    def stageA(gi, h):
                        w0, nwin = GROUPS[gi]
                        plen = nwin * N
                        ets = []
                        for st_s in range(2):
                            st = psBig.tile([128, 392], F32, tag="big")
                            for wi in range(nwin):
                                w = w0 + wi
                                nc.tensor.matmul(
                                    st[0:98, wi * 196:(wi + 1) * 196],
                                    kTa[h][:, w * N + 98 * st_s:w * N + 98 * (st_s + 1)],
                                    qTa[h][:, w * N:(w + 1) * N],
                                    start=True, stop=True)
                            et = pDe.tile([98, 392], BF16, tag="et")
                            nc.scalar.activation(out=et[:, 0:plen],
                                                 in_=st[0:98, 0:plen],
                                                 func=AF.Exp, bias=0.0, scale=1.0)
                            ets.append(et)
                        return ets

                    def stageB(gi, h, ets):
                        w0, nwin = GROUPS[gi]
                        plen = nwin * N
                        _, vt, attnT = gstate[gi]
                        rzb = psBig.tile([128, 392], F32, tag="big")
                        for st_s in range(2):
                            nc.tensor.matmul(rzb[0:1, 0:plen], ones98[:],
                                             ets[st_s][:, 0:plen],
                                             start=(st_s == 0), stop=(st_s == 1))
                        rz = pD.tile([1, 392], BF16, tag="rz")
                        with nc.allow_low_precision(reason="softmax 1/Z in bf16"):
                            nc.vector.reciprocal(out=rz[:, 0:plen],
                                                 in_=rzb[0:1, 0:plen])
                        oT = psO.tile([66, 392], F32, tag="oT")
                        for wi in range(nwin):
                            for st_s in range(2):
                                nc.tensor.matmul(
                                    oT[:, wi * 196:(wi + 1) * 196],
                                    vt[2 * wi + st_s][:, h * 66:(h + 1) * 66],
                                    ets[st_s][:, wi * 196:(wi + 1) * 196],
                                    start=(st_s == 0), stop=(st_s == 1))
                        rzs = pD.tile([64, 392], BF16, tag="rzs")
                        nc.gpsimd.partition_broadcast(rzs[:, 0:plen], rz[:, 0:plen])
                        nc.vector.tensor_tensor(
                            out=attnT[(h % 2) * 64:(h % 2) * 64 + 64, h // 2, 0:plen],
                            in0=oT[0:64, 0:plen], in1=rzs[:, 0:plen], op=ALU.mult)

                    def projpart(gi):
                        w0, nwin = GROUPS[gi]
                        plen = nwin * N
                        _, vt, attnT = gstate[gi]
                        yT = []
                        for m in range(3):
                            pj = psBig.tile([128, 392], F32, tag="big")
                            nc.tensor.matmul(pj[:, 0:plen],
                                             wp_t[:, 0:2, m * 128:(m + 1) * 128],
                                             attnT[:, 0:2, 0:plen],
                                             start=True, stop=False,
                                             perf_mode=mybir.MatmulPerfMode.DoubleRow)
                            nc.tensor.matmul(pj[:, 0:plen],
                                             wp_t[:, 2, m * 128:(m + 1) * 128],
                                             attnT[:, 2, 0:plen],
                                             start=False, stop=True)
                            yTm = pDy.tile([128, 392], BF16, tag=f"yT{m}")
                            if m == 1:
                                nc.vector.tensor_scalar(out=yTm[:, 0:plen],
                                                        in0=pj[:, 0:plen],
                                                        scalar1=bp_t[:, m:m + 1],
                                                        scalar2=None, op0=ALU.add)
                            else:
                                nc.scalar.activation(out=yTm[:, 0:plen], in_=pj[:, 0:plen],
                                                     func=AF.Identity,
                                                     bias=bp_t[:, m:m + 1], scale=1.0)
                            yT.append(yTm)
                        for st_s in range(2 * nwin):
                            pt = psT.tile([128, DIM], BF16, tag="tr")
                            for m in range(3):
                                nc.tensor.transpose(pt[0:98, m * 128:(m + 1) * 128],
                                                    yT[m][:, st_s * 98:(st_s + 1) * 98],
                                                    ident[:])
                            ysb = pD.tile([98, DIM], FP8, tag="ysb")
                            if st_s % 2 == 0:
                                nc.vector.tensor_copy(out=ysb[:], in_=pt[0:98, :])
                            else:
                                nc.scalar.copy(out=ysb[:], in_=pt[0:98, :])
                            w = w0 + st_s // 2
                            i0 = (w // 5) * 14 + (st_s % 2) * 7
                            j0 = (w % 5) * 14
                            na = min(7, max(0, 64 - i0))
                            if na == 0:
                                continue
                            ya = y_d.ap()
                            ydst = bass.AP(tensor=ya.tensor,
                                           offset=(i0 * GRID + j0) * DIM,
                                           ap=[[GRID * DIM, na], [DIM, 14], [1, DIM]])
                            nc.gpsimd.dma_start(out=ydst, in_=ysb[0:na * 14, :])

                    for gi in range(NG + 1):
                        if gi < NG:
                            w0, nwin = GROUPS[gi]
                            g0 = w0 * N
                            vt = [pDv.tile([98, NH * 66], BF16, tag=f"vt{s}",
                                           name=f"vt{s}") for s in range(2 * nwin)]
                            for s in range(2 * nwin):
                                nc.sync.dma_start(
                                    out=vt[s][:],
                                    in_=v_d[g0 + 98 * s:g0 + 98 * (s + 1), :])
                            attnT = pDa.tile([128, 3, 392], FP8, tag="attnT",
                                             name="attnT")
                            gstate[gi] = (nwin * N, vt, attnT)
                            for h in range(NH):
                                ets = stageA(gi, h)
                                if pend:
                                    stageB(*pend.pop(0))
                                pend.append((gi, h, ets))
                                if h == 0 and gi >= 1:
                                    projpart(gi - 1)
                                    for eg in E_AFTER.get(gi - 1, []):
                                        emit_E(eg)
                        else:
                            while pend:
                                stageB(*pend.pop(0))
                            projpart(NG - 1)
                            for eg in E_AFTER.get(NG - 1, []):
                                emit_E(eg)

    nc.compile()
    return nc


_NC = None


def _get_nc():
    global _NC
    if _NC is None:
        _NC = build_bass()
    return _NC


def _host_prep(inputs):
    f = np.float32
    ln1_w = np.asarray(inputs["ln1_w"], f); ln1_b = np.asarray(inputs["ln1_b"], f)
    qkv_w = np.asarray(inputs["qkv_w"], f); qkv_b = np.asarray(inputs["qkv_b"], f)
    proj_w = np.asarray(inputs["proj_w"], f); proj_b = np.asarray(inputs["proj_b"], f)
    ln2_w = np.asarray(inputs["ln2_w"], f); ln2_b = np.asarray(inputs["ln2_b"], f)
    fc1_w = np.asarray(inputs["fc1_w"], f); fc1_b = np.asarray(inputs["fc1_b"], f)
    fc2_w = np.asarray(inputs["fc2_w"], f); fc2_b = np.asarray(inputs["fc2_b"], f)
    rel_h = np.asarray(inputs["rel_pos_h"], f); rel_w = np.asarray(inputs["rel_pos_w"], f)

    Wq = (ln1_w[:, None] * qkv_w[:, 0:DIM]) * SCALE
    Wk = ln1_w[:, None] * qkv_w[:, DIM:2 * DIM]
    bq_full = (ln1_b @ qkv_w[:, 0:DIM] + qkv_b[0:DIM]) * SCALE
    Wv = ln1_w[:, None] * qkv_w[:, 2 * DIM:]
    bv = ln1_b @ qkv_w[:, 2 * DIM:] + qkv_b[2 * DIM:]

    # per-head blocks [q_h | k_h]
    wqk = np.zeros((DIM, 2 * DIM), f)
    bq = np.zeros((NH * 128,), f)
    for h in range(NH):
        wqk[:, h * 128:h * 128 + 64] = Wq[:, h * 64:(h + 1) * 64]
        wqk[:, h * 128 + 64:h * 128 + 128] = Wk[:, h * 64:(h + 1) * 64]
        bq[h * 128:h * 128 + 64] = bq_full[h * 64:(h + 1) * 64]

    coords = np.arange(WS)[:, None] - np.arange(WS)[None, :] + (WS - 1)
    Rh = rel_h[coords]   # [q_row, k_row, c]
    Rw = rel_w[coords]
    relm = np.zeros((128, 2 * N), f)
    for r in range(14):
        relm[64:128, r * 14:(r + 1) * 14] = Rh[r].T / SCALE
    for c in range(14):
        relm[64:128, 196 + c * 14:196 + (c + 1) * 14] = Rw[c].T / SCALE

    # kinit: kpat rows (rel broadcast one-hots), zeros elsewhere
    kinit = np.zeros((64, NTOK), f)
    for w in range(NW):
        for a in range(14):
            for b in range(14):
                col = w * N + a * 14 + b
                kinit[a, col] = 1.0        # key-row one-hot (pairs with relh)
                kinit[32 + b, col] = 1.0   # key-col one-hot (pairs with relw)

    return {
        "wqk": wqk.astype(BF), "bq": bq,
        "wv": np.ascontiguousarray(Wv).astype(BF),
        "relm": relm.astype(BF),
        "kinit": kinit.astype(BF),
        "qinit": np.zeros((64, NTOK), BF),
        "wp": np.ascontiguousarray(proj_w).astype(F8),
        "bp": np.ascontiguousarray(proj_b + bv @ proj_w, f),
        "w1": np.ascontiguousarray(ln2_w[:, None] * fc1_w).astype(F8),
        "b1": np.ascontiguousarray(ln2_b @ fc1_w + fc1_b, f),
        "w2": np.ascontiguousarray(fc2_w).astype(F8),
        "b2": np.ascontiguousarray(fc2_b, f),
    }


def kernel(**inputs):
    nc = _get_nc()
    shared = _host_prep(inputs)
    x = np.asarray(inputs["x"], np.float32).reshape(B, NVAL, DIM)
    in_maps = [dict(shared, x=np.ascontiguousarray(x[c])) for c in range(B)]
    res = run_bass_kernel_spmd(nc, in_maps, list(range(B)))
    out = np.stack([res.results[c]["out"] for c in range(B)])
    return out.reshape(B, H, W, DIM)


if __name__ == "__main__":
    build_bass()
    print("build ok")


# revision 3
# speedup vs baseline: 1.0274x; 1.0274x over previous
"""Bass/Trainium2 kernel for nn_BlockForNormalWindow (windowed-attention
transformer block), data-parallel over batch across 8 NeuronCores.

v2: bf16 compute, SBUF-resident composite qTa/kTa score operands
(rel-pos rows + q/k rows stacked on the contract dim), per-window
score/AV matmuls, software-pipelined phase interleaving (qkv+rel per
head; MLP groups interleaved into attention groups).
"""
import sys
sys.path.insert(0, '/opt/trn_rl_repo')

import numpy as np
import ml_dtypes
import concourse.bass as bass
import concourse.mybir as mybir
import concourse.tile as tile
from concourse import bacc
from concourse.bass_utils import run_bass_kernel_spmd
from concourse.masks import make_identity

F32 = mybir.dt.float32
BF16 = mybir.dt.bfloat16
FP8 = mybir.dt.float8e4
AF = mybir.ActivationFunctionType
ALU = mybir.AluOpType
BF = ml_dtypes.bfloat16
F8 = ml_dtypes.float8_e4m3

B, H, W = 8, 64, 64
DIM, NH, WS = 384, 6, 14
HD = DIM // NH               # 64
MLP = 4 * DIM                # 1536
EPS = 1e-5
SCALE = HD ** -0.5
GRID = 70                    # padded grid side
NW = 25                      # windows
N = WS * WS                  # 196 tokens per window
NTOK = NW * N                # 4900
NVAL = H * W                 # 4096

# window groups for qkv/attention: 12 groups of 2 windows + 1 single
GROUPS = [(2 * g, 2) for g in range(12)] + [(24, 1)]
# E-group eg emitted after D-group g
E_AFTER = {2: [0], 5: [1, 2], 7: [3, 4], 9: [5, 6], 12: [7]}
# v-chunk ranges computed per BC head iteration
VC_PER_H = [9, 9, 8, 8, 8, 8]


def _win_origin(w):
    return (w // 5) * 14 * GRID + (w % 5) * 14


def _ap(t, offset_elems, dims):
    """AP at partition range of t (sliced), offset in elements, free dims."""
    a = t[:, 0:1]
    return bass.AP(tensor=a.tensor, offset=a.offset + offset_elems,
                   ap=[a.ap[0]] + dims)


def _app(t, p0, p1, offset_elems, dims):
    """Like _ap but with partition slice [p0:p1]."""
    a = t[p0:p1, 0:1]
    return bass.AP(tensor=a.tensor, offset=a.offset + offset_elems,
                   ap=[a.ap[0]] + dims)


def build_bass():
    nc = bacc.Bacc("TRN2", target_bir_lowering=False, debug=False)

    x_in = nc.dram_tensor("x", [NVAL, DIM], F32, kind="ExternalInput")
    wqk_in = nc.dram_tensor("wqk", [DIM, 2 * DIM], BF16, kind="ExternalInput")
    bq_in = nc.dram_tensor("bq", [NH * 128], F32, kind="ExternalInput")
    wv_in = nc.dram_tensor("wv", [DIM, DIM], BF16, kind="ExternalInput")
    relm_in = nc.dram_tensor("relm", [128, 2 * N], BF16, kind="ExternalInput")
    kinit_in = nc.dram_tensor("kinit", [64, NTOK], BF16, kind="ExternalInput")
    qinit_in = nc.dram_tensor("qinit", [64, NTOK], BF16, kind="ExternalInput")
    wp_in = nc.dram_tensor("wp", [DIM, DIM], FP8, kind="ExternalInput")
    bp_in = nc.dram_tensor("bp", [DIM], F32, kind="ExternalInput")
    w1_in = nc.dram_tensor("w1", [DIM, MLP], FP8, kind="ExternalInput")
    b1_in = nc.dram_tensor("b1", [MLP], F32, kind="ExternalInput")
    w2_in = nc.dram_tensor("w2", [MLP, DIM], FP8, kind="ExternalInput")
    b2_in = nc.dram_tensor("b2", [DIM], F32, kind="ExternalInput")
    out_d = nc.dram_tensor("out", [NVAL, DIM], F32, kind="ExternalOutput")

    v_d = nc.dram_tensor("v_d", [NTOK, NH * 66], BF16)
    y_d = nc.dram_tensor("y_d", [64 * GRID, DIM], BF16)

    with tile.TileContext(nc) as tc:
      with tc.tile_pool(name="singles", bufs=1) as singles:
        ident_f = singles.tile([128, 128], F32)
        make_identity(nc, ident_f[:])
        ident = singles.tile([128, 128], BF16)
        nc.vector.tensor_copy(out=ident[:], in_=ident_f[:])

        eps_t = singles.tile([128, 1], F32)
        nc.vector.memset(eps_t[:], EPS)
        ones64 = singles.tile([1, 64], BF16)
        nc.vector.memset(ones64[:], 1.0)
        ones98 = singles.tile([98, 1], BF16)
        nc.vector.memset(ones98[:], 1.0)

        bq_t = singles.tile([128, NH], F32)
        bp_t = singles.tile([128, 3], F32)
        b1_t = singles.tile([128, 12], F32)
        b2_t = singles.tile([128, 3], F32)
        wqk_t = singles.tile([128, 3, 2 * DIM], BF16)
        wv_t = singles.tile([128, 3, DIM], BF16)
        wp_t = singles.tile([128, 3, DIM], FP8)
        relm_t = singles.tile([128, 2 * N], BF16)

        # resident composite score operands per head:
        #   rows 0-13 relh/kpat-h, 14-31 zero, 32-45 relw/kpat-w, 46-63 zero,
        #   rows 64-127 q/k
        with tc.tile_pool(name="pQK", bufs=1) as pQK:
            qTa = [pQK.tile([128, NTOK], BF16, tag=f"qTa{h}", name=f"qTa{h}")
                   for h in range(NH)]
            kTa = [pQK.tile([128, NTOK], BF16, tag=f"kTa{h}", name=f"kTa{h}")
                   for h in range(NH)]
            def load_singles():
                nc.sync.dma_start(out=bq_t[:], in_=bq_in.ap().rearrange("(m p) -> p m", p=128))
                nc.sync.dma_start(out=bp_t[:], in_=bp_in.ap().rearrange("(m p) -> p m", p=128))
                nc.sync.dma_start(out=b1_t[:], in_=b1_in.ap().rearrange("(m p) -> p m", p=128))
                nc.sync.dma_start(out=b2_t[:], in_=b2_in.ap().rearrange("(m p) -> p m", p=128))
                nc.sync.dma_start(out=wqk_t[:], in_=wqk_in.ap().rearrange("(kc p) n -> p kc n", p=128))
                nc.sync.dma_start(out=wv_t[:], in_=wv_in.ap().rearrange("(kc p) n -> p kc n", p=128))
                nc.sync.dma_start(out=wp_t[:], in_=wp_in.ap().rearrange("(kc p) n -> p kc n", p=128))
                nc.sync.dma_start(out=relm_t[:], in_=relm_in.ap())
                for h in range(NH):
                    nc.sync.dma_start(out=qTa[h][0:64, :], in_=qinit_in.ap())
                    nc.sync.dma_start(out=kTa[h][0:64, :], in_=kinit_in.ap())

            # ---- Phase A: LN1 + transpose into hT, then B/C per head ----
            with tc.tile_pool(name="pHT", bufs=1) as pHT:
                hT = pHT.tile([128, 3 * NTOK], BF16, tag="hT", name="hT")
                # zero the pad region of the 70x70 grid
                for kc in range(3):
                    nc.gpsimd.memset(_ap(hT, kc * NTOK + 64 * GRID,
                                         [[1, NTOK - 64 * GRID]]), 0.0)
                    nc.gpsimd.memset(_ap(hT, kc * NTOK + 64,
                                         [[GRID, 64], [1, 6]]), 0.0)

                with tc.tile_pool(name="pA", bufs=6) as pA, \
                     tc.tile_pool(name="pA_ps", bufs=2, space="PSUM") as pA_ps:
                    for t in range(32):
                        xt = pA.tile([128, DIM], F32, tag="xt")
                        nc.sync.dma_start(out=xt[:], in_=x_in[t * 128:(t + 1) * 128, :])
                        stats = pA.tile([128, 6], F32, tag="st")
                        nc.vector.bn_stats(out=stats[:], in_=xt[:])
                        mv = pA.tile([128, 2], F32, tag="mv")
                        nc.vector.bn_aggr(out=mv[:], in_=stats[:])
                        rstd = pA.tile([128, 1], F32, tag="rstd")
                        nc.scalar.activation(out=rstd[:], in_=mv[:, 1:2], func=AF.Sqrt,
                                             bias=eps_t[:], scale=1.0)
                        nc.vector.reciprocal(out=rstd[:], in_=rstd[:])
                        nmr = pA.tile([128, 1], F32, tag="nmr")
                        nc.vector.scalar_tensor_tensor(out=nmr[:], in0=mv[:, 0:1],
                                                       scalar=-1.0, in1=rstd[:],
                                                       op0=ALU.mult, op1=ALU.mult)
                        hn = pA.tile([128, DIM], BF16, tag="hn")
                        nc.scalar.activation(out=hn[:], in_=xt[:], func=AF.Identity,
                                             bias=nmr[:], scale=rstd[:])
                        pt = pA_ps.tile([128, DIM], BF16, tag="tra")
                        for c in range(3):
                            nc.tensor.transpose(pt[:, c * 128:(c + 1) * 128],
                                                hn[:, c * 128:(c + 1) * 128], ident[:])
                        dst = _ap(hT, 2 * t * GRID, [[NTOK, 3], [GRID, 2], [1, 64]])
                        if t % 2 == 0:
                            nc.vector.tensor_copy(out=dst, in_=pt[:])
                        else:
                            nc.scalar.copy(out=dst, in_=pt[:])

                load_singles()

                # ---- Phase B+C: qkv + rel, per head, v interleaved ----
                vchunk = 0
                with tc.tile_pool(name="pB", bufs=5) as pB, \
                     tc.tile_pool(name="pB_ps", bufs=2, space="PSUM") as pB_ps, \
                     tc.tile_pool(name="pC_ps", bufs=3, space="PSUM") as pC_ps, \
                     tc.tile_pool(name="pV_ps", bufs=2, space="PSUM") as pV_ps:
                    for h in range(NH):
                        # qk for all groups
                        for (w0, nwin) in GROUPS:
                            plen = nwin * N
                            g0 = w0 * N
                            o0 = _win_origin(w0)
                            if nwin == 2:
                                dims = [[_win_origin(w0 + 1) - o0, 2], [GRID, 14], [1, 14]]
                            else:
                                dims = [[GRID, 14], [1, 14]]
                            ps = pB_ps.tile([128, 392], F32, tag="qk")
                            for kc in range(3):
                                nc.tensor.matmul(ps[:, 0:plen],
                                                 wqk_t[:, kc, h * 128:(h + 1) * 128],
                                                 _ap(hT, kc * NTOK + o0, dims),
                                                 start=(kc == 0), stop=(kc == 2))
                            # k rows 64-127 -> kTa (same partitions, Act)
                            nc.scalar.copy(out=kTa[h][64:128, g0:g0 + plen],
                                           in_=ps[64:128, 0:plen])
                            # q rows 0-63 + bias -> qTa rows 64-127 (DVE shift)
                            nc.vector.tensor_scalar(out=qTa[h][64:128, g0:g0 + plen],
                                                    in0=ps[0:64, 0:plen],
                                                    scalar1=bq_t[0:64, h:h + 1],
                                                    scalar2=None, op0=ALU.add)
                        # rel rows for this head
                        for rc in range(14):
                            cps = pC_ps.tile([46, 350], F32, tag="c")
                            nc.tensor.matmul(cps[0:14, :],
                                             relm_t[64:128, rc * 14:(rc + 1) * 14],
                                             _app(qTa[h], 64, 128, rc * 14,
                                                  [[196, 25], [1, 14]]),
                                             start=True, stop=True)
                            nc.tensor.matmul(cps[32:46, :],
                                             relm_t[64:128, 196 + rc * 14:196 + (rc + 1) * 14],
                                             _app(qTa[h], 64, 128, rc,
                                                  [[196, 25], [14, 14]]),
                                             start=True, stop=True)
                            dsth = _app(qTa[h], 0, 14, rc * 14, [[196, 25], [1, 14]])
                            dstw = _app(qTa[h], 32, 46, rc, [[196, 25], [14, 14]])
                            if rc % 2 == 0:
                                nc.scalar.copy(out=dsth, in_=cps[0:14, :])
                                nc.vector.tensor_copy(out=dstw, in_=cps[32:46, :])
                            else:
                                nc.vector.tensor_copy(out=dsth, in_=cps[0:14, :])
                                nc.scalar.copy(out=dstw, in_=cps[32:46, :])
                        # v chunks interleaved
                        for _ in range(VC_PER_H[h]):
                            c = vchunk
                            vchunk += 1
                            w = c // 2
                            ov = _win_origin(w) + (c % 2) * 7 * GRID
                            vps = pV_ps.tile([98, DIM], F32, tag="v")
                            hstage = pB.tile([128, 3, 98], BF16, tag="hstage")
                            for kc in range(3):
                                nc.gpsimd.tensor_copy(
                                    out=hstage[:, kc, :],
                                    in_=_ap(hT, kc * NTOK + ov, [[GRID, 7], [1, 14]]))
                            for kc in range(3):
                                nc.tensor.matmul(vps[:], hstage[:, kc, :],
                                                 wv_t[:, kc, :],
                                                 start=(kc == 0), stop=(kc == 2))
                            vsb = pB.tile([98, NH * 66], BF16, tag="vsb")
                            if c < 3:
                                nc.vector.memset(_ap(vsb, 64, [[66, 6], [1, 1]]), 1.0)
                                nc.vector.memset(_ap(vsb, 65, [[66, 6], [1, 1]]), 0.0)
                            dstv = _ap(vsb, 0, [[66, 6], [1, 64]])
                            if c % 2 == 0:
                                nc.scalar.copy(out=dstv, in_=vps[:])
                            else:
                                nc.vector.tensor_copy(out=dstv, in_=vps[:])
                            nc.sync.dma_start(out=v_d[c * 98:(c + 1) * 98, :], in_=vsb[:])

            # ---- Phase D+E interleaved ----
            with tc.tile_pool(name="pW2", bufs=1) as pW2:
                w1_t = pW2.tile([128, 3, MLP], FP8)
                nc.sync.dma_start(out=w1_t[:], in_=w1_in.ap().rearrange("(kc p) n -> p kc n", p=128))
                w2_t = pW2.tile([128, 12, DIM], FP8)
                nc.sync.dma_start(out=w2_t[:], in_=w2_in.ap().rearrange("(kc p) n -> p kc n", p=128))

                with tc.tile_pool(name="pD", bufs=3) as pD, \
                     tc.tile_pool(name="pDv", bufs=3) as pDv, \
                     tc.tile_pool(name="pDa", bufs=2) as pDa, \
                     tc.tile_pool(name="pDe", bufs=6) as pDe, \
                     tc.tile_pool(name="pDy", bufs=2) as pDy, \
                     tc.tile_pool(name="pE", bufs=5) as pE, \
                     tc.tile_pool(name="pEz", bufs=2) as pEz, \
                     tc.tile_pool(name="pEh", bufs=1) as pEh, \
                     tc.tile_pool(name="pEg", bufs=1) as pEg, \
                     tc.tile_pool(name="psBig", bufs=3, space="PSUM") as psBig, \
                     tc.tile_pool(name="psO", bufs=1, space="PSUM") as psO, \
                     tc.tile_pool(name="psT", bufs=1, space="PSUM") as psT, \
                     tc.tile_pool(name="psF", bufs=3, space="PSUM") as psF:

                    def emit_E(eg):
                        zts = []
                        h2T = pEh.tile([128, 3 * 512], FP8, tag="h2T")
                        for tt in range(4):
                            t = eg * 4 + tt
                            xe = pE.tile([128, DIM], F32, tag="xe")
                            nc.sync.dma_start(out=xe[:], in_=x_in[t * 128:(t + 1) * 128, :])
                            ye = pE.tile([128, DIM], BF16, tag="ye")
                            ya2 = y_d.ap()
                            ysrc2 = bass.AP(tensor=ya2.tensor,
                                            offset=2 * t * GRID * DIM,
                                            ap=[[GRID * DIM, 2], [DIM, 64], [1, DIM]])
                            nc.sync.dma_start(out=ye[:], in_=ysrc2)
                            ze = pEz.tile([128, DIM], F32, tag=f"z{tt}")
                            nc.vector.tensor_tensor(out=ze[:], in0=xe[:], in1=ye[:],
                                                    op=ALU.add)
                            zts.append(ze)
                            stats = pE.tile([128, 6], F32, tag="ste")
                            nc.vector.bn_stats(out=stats[:], in_=ze[:])
                            mv = pE.tile([128, 2], F32, tag="mve")
                            nc.vector.bn_aggr(out=mv[:], in_=stats[:])
                            rstd = pE.tile([128, 1], F32, tag="rstde")
                            nc.scalar.activation(out=rstd[:], in_=mv[:, 1:2], func=AF.Sqrt,
                                                 bias=eps_t[:], scale=1.0)
                            nc.vector.reciprocal(out=rstd[:], in_=rstd[:])
                            nmr = pE.tile([128, 1], F32, tag="nmre")
                            nc.vector.scalar_tensor_tensor(out=nmr[:], in0=mv[:, 0:1],
                                                           scalar=-1.0, in1=rstd[:],
                                                           op0=ALU.mult, op1=ALU.mult)
                            hn = pE.tile([128, DIM], BF16, tag="hne")
                            nc.scalar.activation(out=hn[:], in_=ze[:], func=AF.Identity,
                                                 bias=nmr[:], scale=rstd[:])
                            pt = psT.tile([128, DIM], BF16, tag="tr")
                            for c in range(3):
                                nc.tensor.transpose(pt[:, c * 128:(c + 1) * 128],
                                                    hn[:, c * 128:(c + 1) * 128], ident[:])
                            dst = _ap(h2T, tt * 128, [[512, 3], [1, 128]])
                            nc.vector.tensor_copy(out=dst, in_=pt[:])
                        gt = pEg.tile([128, 12, 512], FP8, tag="gt")
                        for m in range(12):
                            f1 = psF.tile([128, 512], F32, tag="fc")
                            nc.tensor.matmul(f1[:], w1_t[:, 0:2, m * 128:(m + 1) * 128],
                                             _ap(h2T, 0, [[512, 2], [1, 512]]),
                                             start=True, stop=False,
                                             perf_mode=mybir.MatmulPerfMode.DoubleRow)
                            nc.tensor.matmul(f1[:], w1_t[:, 2, m * 128:(m + 1) * 128],
                                             _ap(h2T, 2 * 512, [[1, 512]]),
                                             start=False, stop=True)
                            nc.scalar.activation(out=gt[:, m, :], in_=f1[:], func=AF.Gelu,
                                                 bias=b1_t[:, m:m + 1], scale=1.0)
                        o2 = []
                        for m in range(3):
                            f2 = psF.tile([128, 512], F32, tag="fc")
                            for kc in range(6):
                                nc.tensor.matmul(f2[:], w2_t[:, 2 * kc:2 * kc + 2, m * 128:(m + 1) * 128],
                                                 gt[:, 2 * kc:2 * kc + 2, :],
                                                 start=(kc == 0), stop=(kc == 5),
                                                 perf_mode=mybir.MatmulPerfMode.DoubleRow)
                            om = pEh.tile([128, 512], BF16, tag=f"o2T{m}")
                            nc.vector.tensor_scalar(out=om[:], in0=f2[:],
                                                    scalar1=b2_t[:, m:m + 1],
                                                    scalar2=None, op0=ALU.add)
                            o2.append(om)
                        for tt in range(4):
                            t = eg * 4 + tt
                            pt = psT.tile([128, DIM], BF16, tag="tr")
                            for c in range(3):
                                nc.tensor.transpose(pt[:, c * 128:(c + 1) * 128],
                                                    o2[c][:, tt * 128:(tt + 1) * 128],
                                                    ident[:])
                            oe = pE.tile([128, DIM], F32, tag="oe")
                            nc.vector.tensor_tensor(out=oe[:], in0=zts[tt][:],
                                                    in1=pt[:], op=ALU.add)
                            nc.sync.dma_start(out=out_d[t * 128:(t + 1) * 128, :], in_=oe[:])

                    # software-pipelined D: stage A(g,h) = st+exp; stage B(g,h)
                    # = Z/recip/AV/bcast/mult one head behind; proj one group
                    # behind, so the in-order PE never waits on Act/DVE.
                    NG = len(GROUPS)
                    pend = []       # (gi, h, ets, vt, attnT) awaiting stage B
                    gstate = {}     # gi -> (plen, vt, attnT)

                    def stageA(gi, h):
                        w0, nwin = GROUPS[gi]
                        plen = nwin * N
                        ets = []
                        for st_s in range(2):
                            st = psBig.tile([128, 392], F32, tag="big")
                            for wi in range(nwin):
                                w = w0 + wi
                                nc.tensor.matmul(
                                    st[0:98, wi * 196:(wi + 1) * 196],
                                    kTa[h][:, w * N + 98 * st_s:w * N + 98 * (st_s + 1)],
                                    qTa[h][:, w * N:(w + 1) * N],
                                    start=True, stop=True)
                            et = pDe.tile([98, 392], BF16, tag="et")
                            nc.scalar.activation(out=et[:, 0:plen],
                                                 in_=st[0:98, 0:plen],
                                                 func=AF.Exp, bias=0.0, scale=1.0)
                            ets.append(et)
                        return ets

                    def stageB(gi, h, ets):
                        w0, nwin = GROUPS[gi]
                        plen = nwin * N
                        _, vt, attnT = gstate[gi]
                        rzb = psBig.tile([128, 392], F32, tag="big")
                        for st_s in range(2):
                            nc.tensor.matmul(rzb[0:1, 0:plen], ones98[:],
                                             ets[st_s][:, 0:plen],
                                             start=(st_s == 0), stop=(st_s == 1))
                        rz = pD.tile([1, 392], BF16, tag="rz")
                        with nc.allow_low_precision(reason="softmax 1/Z in bf16"):
                            nc.vector.reciprocal(out=rz[:, 0:plen],
                                                 in_=rzb[0:1, 0:plen])
                        oT = psO.tile([66, 392], F32, tag="oT")
                        for wi in range(nwin):
                            for st_s in range(2):
                                nc.tensor.matmul(
                                    oT[:, wi * 196:(wi + 1) * 196],
                                    vt[2 * wi + st_s][:, h * 66:(h + 1) * 66],
                                    ets[st_s][:, wi * 196:(wi + 1) * 196],
                                    start=(st_s == 0), stop=(st_s == 1))
                        rzs = pD.tile([64, 392], BF16, tag="rzs")
                        nc.gpsimd.partition_broadcast(rzs[:, 0:plen], rz[:, 0:plen])
                        nc.vector.tensor_tensor(
                            out=attnT[(h % 2) * 64:(h % 2) * 64 + 64, h // 2, 0:plen],
                            in0=oT[0:64, 0:plen], in1=rzs[:, 0:plen], op=ALU.mult)

                    def projpart(gi):
                        w0, nwin = GROUPS[gi]
                        plen = nwin * N
                        _, vt, attnT = gstate[gi]
                        yT = []
                        for m in range(3):
                            pj = psBig.tile([128, 392], F32, tag="big")
                            nc.tensor.matmul(pj[:, 0:plen],
                                             wp_t[:, 0:2, m * 128:(m + 1) * 128],
                                             attnT[:, 0:2, 0:plen],
                                             start=True, stop=False,
                                             perf_mode=mybir.MatmulPerfMode.DoubleRow)
                            nc.tensor.matmul(pj[:, 0:plen],
                                             wp_t[:, 2, m * 128:(m + 1) * 128],
                                             attnT[:, 2, 0:plen],
                                             start=False, stop=True)
                            yTm = pDy.tile([128, 392], BF16, tag=f"yT{m}")
                            if m == 1:
                                nc.vector.tensor_scalar(out=yTm[:, 0:plen],
                                                        in0=pj[:, 0:plen],
                                                        scalar1=bp_t[:, m:m + 1],
                                                        scalar2=None, op0=ALU.add)
                            else:
                                nc.scalar.activation(out=yTm[:, 0:plen], in_=pj[:, 0:plen],
                                                     func=AF.Identity,
                                                     bias=bp_t[:, m:m + 1], scale=1.0)
                            yT.append(yTm)
                        for st_s in range(2 * nwin):
                            pt = psT.tile([128, DIM], BF16, tag="tr")
                            for m in range(3):
                                nc.tensor.transpose(pt[0:98, m * 128:(m + 1) * 128],
                                                    yT[m][:, st_s * 98:(st_s + 1) * 98],
                                                    ident[:])
                            ysb = pD.tile([98, DIM], BF16, tag="ysb")
                            if st_s % 2 == 0:
                                nc.vector.tensor_copy(out=ysb[:], in_=pt[0:98, :])
                            else:
                                nc.scalar.copy(out=ysb[:], in_=pt[0:98, :])
                            w = w0 + st_s // 2
                            i0 = (w // 5) * 14 + (st_s % 2) * 7
                            j0 = (w % 5) * 14
                            na = min(7, max(0, 64 - i0))
                            if na == 0:
                                continue
                            ya = y_d.ap()
                            ydst = bass.AP(tensor=ya.tensor,
                                           offset=(i0 * GRID + j0) * DIM,
                                           ap=[[GRID * DIM, na], [DIM, 14], [1, DIM]])
                            nc.gpsimd.dma_start(out=ydst, in_=ysb[0:na * 14, :])

                    for gi in range(NG + 1):
                        if gi < NG:
                            w0, nwin = GROUPS[gi]
                            g0 = w0 * N
                            vt = [pDv.tile([98, NH * 66], BF16, tag=f"vt{s}",
                                           name=f"vt{s}") for s in range(2 * nwin)]
                            for s in range(2 * nwin):
                                nc.sync.dma_start(
                                    out=vt[s][:],
                                    in_=v_d[g0 + 98 * s:g0 + 98 * (s + 1), :])
                            attnT = pDa.tile([128, 3, 392], FP8, tag="attnT",
                                             name="attnT")
                            gstate[gi] = (nwin * N, vt, attnT)
                            for h in range(NH):
                                ets = stageA(gi, h)
                                if pend:
                                    stageB(*pend.pop(0))
                                pend.append((gi, h, ets))
                                if h == 0 and gi >= 1:
                                    projpart(gi - 1)
                                    for eg in E_AFTER.get(gi - 1, []):
                                        emit_E(eg)
                        else:
                            while pend:
                                stageB(*pend.pop(0))
                            projpart(NG - 1)
                            for eg in E_AFTER.get(NG - 1, []):
                                emit_E(eg)

    nc.compile()
    return nc


_NC = None


def _get_nc():
    global _NC
    if _NC is None:
        _NC = build_bass()
    return _NC


def _host_prep(inputs):
    f = np.float32
    ln1_w = np.asarray(inputs["ln1_w"], f); ln1_b = np.asarray(inputs["ln1_b"], f)
    qkv_w = np.asarray(inputs["qkv_w"], f); qkv_b = np.asarray(inputs["qkv_b"], f)
    proj_w = np.asarray(inputs["proj_w"], f); proj_b = np.asarray(inputs["proj_b"], f)
    ln2_w = np.asarray(inputs["ln2_w"], f); ln2_b = np.asarray(inputs["ln2_b"], f)
    fc1_w = np.asarray(inputs["fc1_w"], f); fc1_b = np.asarray(inputs["fc1_b"], f)
    fc2_w = np.asarray(inputs["fc2_w"], f); fc2_b = np.asarray(inputs["fc2_b"], f)
    rel_h = np.asarray(inputs["rel_pos_h"], f); rel_w = np.asarray(inputs["rel_pos_w"], f)

    Wq = (ln1_w[:, None] * qkv_w[:, 0:DIM]) * SCALE
    Wk = ln1_w[:, None] * qkv_w[:, DIM:2 * DIM]
    bq_full = (ln1_b @ qkv_w[:, 0:DIM] + qkv_b[0:DIM]) * SCALE
    Wv = ln1_w[:, None] * qkv_w[:, 2 * DIM:]
    bv = ln1_b @ qkv_w[:, 2 * DIM:] + qkv_b[2 * DIM:]

    # per-head blocks [q_h | k_h]
    wqk = np.zeros((DIM, 2 * DIM), f)
    bq = np.zeros((NH * 128,), f)
    for h in range(NH):
        wqk[:, h * 128:h * 128 + 64] = Wq[:, h * 64:(h + 1) * 64]
        wqk[:, h * 128 + 64:h * 128 + 128] = Wk[:, h * 64:(h + 1) * 64]
        bq[h * 128:h * 128 + 64] = bq_full[h * 64:(h + 1) * 64]

    coords = np.arange(WS)[:, None] - np.arange(WS)[None, :] + (WS - 1)
    Rh = rel_h[coords]   # [q_row, k_row, c]
    Rw = rel_w[coords]
    relm = np.zeros((128, 2 * N), f)
    for r in range(14):
        relm[64:128, r * 14:(r + 1) * 14] = Rh[r].T / SCALE
    for c in range(14):
        relm[64:128, 196 + c * 14:196 + (c + 1) * 14] = Rw[c].T / SCALE

    # kinit: kpat rows (rel broadcast one-hots), zeros elsewhere
    kinit = np.zeros((64, NTOK), f)
    for w in range(NW):
        for a in range(14):
            for b in range(14):
                col = w * N + a * 14 + b
                kinit[a, col] = 1.0        # key-row one-hot (pairs with relh)
                kinit[32 + b, col] = 1.0   # key-col one-hot (pairs with relw)

    return {
        "wqk": wqk.astype(BF), "bq": bq,
        "wv": np.ascontiguousarray(Wv).astype(BF),
        "relm": relm.astype(BF),
        "kinit": kinit.astype(BF),
        "qinit": np.zeros((64, NTOK), BF),
        "wp": np.ascontiguousarray(proj_w).astype(F8),
        "bp": np.ascontiguousarray(proj_b + bv @ proj_w, f),
        "w1": np.ascontiguousarray(ln2_w[:, None] * fc1_w).astype(F8),
        "b1": np.ascontiguousarray(ln2_b @ fc1_w + fc1_b, f),
        "w2": np.ascontiguousarray(fc2_w).astype(F8),
        "b2": np.ascontiguousarray(fc2_b, f),
    }


def kernel(**inputs):
    nc = _get_nc()
    shared = _host_prep(inputs)
    x = np.asarray(inputs["x"], np.float32).reshape(B, NVAL, DIM)
    in_maps = [dict(shared, x=np.ascontiguousarray(x[c])) for c in range(B)]
    res = run_bass_kernel_spmd(nc, in_maps, list(range(B)))
    out = np.stack([res.results[c]["out"] for c in range(B)])
    return out.reshape(B, H, W, DIM)


if __name__ == "__main__":
    build_bass()
    print("build ok")
